# revision 12
# baseline (speedup 1.0000x reference)
"""Trainium2 Bass kernel for a 3-layer bidirectional LSTM encoder.

Problem: words [64,512] -> embedding [256] -> 3 x biLSTM(H=512) -> out [64,512,1024].

Sharding (8 NeuronCores): direction x batch-quarter (cores 0-3 forward, 4-7
backward on host-reversed input, 16 sequences each). v2 rewrite driven by the
baseline trace (52.4ms, PE stuck at K=4/8 half clock, fp32 2-pass matmuls at
~427ns each, 2.5us serial elementwise chain per step):

- All matmul operands fp16 (1 PE pass instead of fp32's LOW/HIGH 2-pass, and
  fast-weight-load kicks in). PSUM/c stay fp32.
- The input GEMM is interleaved between recurrence steps (a few fat N=512
  matmuls per step) so the tensor engine never idles long enough for the HAM
  clock gate to drop and the GEMM costs no extra wall time.
- xg lives in an SBUF ring (3 x 32-step blocks) - no DRAM round trip.
- xg enters PSUM through an identity matmul at the head of each accumulation
  group, so the per-step chain starts at the sigmoid (no vector add).
- Per-step chain split into two half-chains (h-chunks 0,1 | 2,3). Each half's
  sigmoid only needs its own 8 gate tiles, and next step's k01 matmuls only
  need half 1's h, so elementwise overlaps the other half's matmuls.
- tanh(g) folded into the big sigmoid via tanh(x) = 2*sigmoid(2x)-1 (g-gate
  weight rows pre-scaled 2x on host); chain ops spread over DVE/ACT/GpSimd.
- Layer exchange: pairwise AllGather (groups [q, q+4]) of fp16 staging planes;
  partner rows pulled per-GEMM-block by indirect DMA gather (host-baked row
  ids keep the SPMD program uniform).
"""

import os
import sys

import numpy as np

for _p in ("/opt/trn_rl_repo", os.path.dirname(os.path.abspath(__file__))):
    if os.path.isdir(_p) and _p not in sys.path:
        sys.path.insert(0, _p)

import bass_rust
import concourse.bass as bass
import concourse.tile as tile
from concourse import bass_utils, mybir

F32 = mybir.dt.float32
F16 = mybir.dt.float16
I32 = mybir.dt.int32
ACTF = mybir.ActivationFunctionType
ALU = mybir.AluOpType

B, T = 64, 512
NWORDS, E, H, L = 32000, 256, 512, 3
G = 4 * H  # 2048 gate rows
NCORES = 8
BLOC = B // 4  # 16 sequences per core
NTIL = G // 128  # 16 gate tiles
SBLK = 32  # recurrence steps per xg ring block (= 512 GEMM columns)

# gate-row permutation to [i f o g] tile blocks (reference order is i, f, g, o)
_PERM = np.concatenate(
    [np.arange(0, 512), np.arange(512, 1024), np.arange(1536, 2048), np.arange(1024, 1536)]
)


def _split_multi_waits(nc, max_waits=1):
    """walrus accepts at most one semaphore sync-wait per instruction; fan
    extra waits out onto same-engine NoOps just before it."""
    n = 0
    for f in nc.m.functions:
        for blk in f.blocks:
            insts = list(blk.instructions)
            out = []
            changed = False
            for inst in insts:
                si = inst.sync_info
                waits = list(si.on_wait) if si is not None else []
                if len(waits) > max_waits:
                    extra, keep = waits[:-max_waits], waits[-max_waits:]
                    for j in range(0, len(extra), max_waits):
                        nop = mybir.InstNoOp(name=f"{inst.name}-wsplit{j}", ins=[], outs=[])
                        nop.engine = inst.engine
                        nop.sync_info = bass_rust.SyncInfo(on_wait=extra[j : j + max_waits], on_update=[])
                        out.append(nop)
                        n += 1
                    inst.sync_info = bass_rust.SyncInfo(on_wait=keep, on_update=list(si.on_update))
                    changed = True
                out.append(inst)
            if changed:
                try:
                    blk.set_instructions(out)
                except Exception:
                    blk.instructions = out
    return n


def _build_nc(t_len=T):
    TB = t_len * BLOC
    NBLK = t_len // SBLK

    nc = bass.Bass(num_devices=NCORES, detect_race_conditions=False, disable_frame_to_traceback=True)

    x0T = nc.dram_tensor("x0T", [2 * 128, TB], F16, kind="ExternalInput")
    wih = [
        nc.dram_tensor("wih0T", [E, G], F16, kind="ExternalInput"),
        nc.dram_tensor("wih1T", [2 * H, G], F16, kind="ExternalInput"),
        nc.dram_tensor("wih2T", [2 * H, G], F16, kind="ExternalInput"),
    ]
    whh = [nc.dram_tensor(f"whh{l}T", [H, G], F16, kind="ExternalInput") for l in range(L)]
    bias = [nc.dram_tensor(f"bias{l}", [128, NTIL], F32, kind="ExternalInput") for l in range(L)]
    pidx = nc.dram_tensor("pidx", [128, 4 * NBLK], I32, kind="ExternalInput")
    ident = nc.dram_tensor("ident", [128, 128], F16, kind="ExternalInput")
    outbuf = nc.dram_tensor("outbuf", [4 * 128, TB], F16, kind="ExternalOutput")

    myh = nc.dram_tensor("myh", [4 * 128, TB], F16, kind="Internal")
    # stage rows = (nb, k, p): each 32-step block is a contiguous [512, 512]
    # region so its AllGather can fire as soon as the block's h is staged.
    stage = [nc.dram_tensor(f"stage{l}", [NBLK * 4 * 128, SBLK * BLOC], F16, kind="Internal") for l in range(2)]
    # agout rows = (nb, slot, k, p)
    agout = [
        nc.dram_tensor(f"agout{l}", [NBLK * 2 * 4 * 128, SBLK * BLOC], F16, kind="Internal")
        for l in range(2)
    ]

    with tile.TileContext(nc) as tc:
        with tc.tile_pool(name="const", bufs=1) as constp:
            pidx_sb = constp.tile([128, 4 * NBLK], I32, tag="pidx")
            nc.sync.dma_start(out=pidx_sb[:], in_=pidx[:])
            id_sb = constp.tile([128, 128], F16, tag="ident")
            nc.sync.dma_start(out=id_sb[:], in_=ident[:])
            x0_sb = constp.tile([128, 2 * TB], F16, tag="x0")
            for k in range(2):
                nc.sync.dma_start(out=x0_sb[:, k * TB : (k + 1) * TB], in_=x0T[k * 128 : (k + 1) * 128, :])
            h0 = constp.tile([128, 64], F16, tag="h0")
            nc.vector.memset(h0[:], 0.0)
            c0 = constp.tile([128, 64], F32, tag="c0")
            nc.vector.memset(c0[:], 0.0)

            for l in range(L):
                KC = 2 if l == 0 else 8
                dst_plane = outbuf if l == L - 1 else myh

                with tc.tile_pool(name=f"w{l}", bufs=1) as wp, \
                     tc.tile_pool(name=f"ring{l}", bufs=3) as ringp, \
                     tc.tile_pool(name=f"xt{l}", bufs=KC + 2) as xtp, \
                     tc.tile_pool(name=f"hh{l}", bufs=2) as hhp, \
                     tc.tile_pool(name=f"ew{l}", bufs=2) as ewp, \
                     tc.tile_pool(name=f"psR{l}", bufs=2, space="PSUM") as psrp, \
                     tc.tile_pool(name=f"psG{l}", bufs=2, space="PSUM") as psgp:

                    wih_sb = wp.tile([128, KC * G], F16, tag="wih")
                    for k in range(KC):
                        nc.sync.dma_start(out=wih_sb[:, k * G : (k + 1) * G], in_=wih[l][k * 128 : (k + 1) * 128, :])
                    whh_sb = wp.tile([128, 4 * G], F16, tag="whh")
                    for k in range(4):
                        nc.sync.dma_start(out=whh_sb[:, k * G : (k + 1) * G], in_=whh[l][k * 128 : (k + 1) * 128, :])
                    bias_sb = wp.tile([128, NTIL], F32, tag="bias")
                    nc.sync.dma_start(out=bias_sb[:], in_=bias[l][:])

                    ring_tiles = {}
                    hh_tiles = {}
                    evac_rr = [nc.vector, nc.vector]

                    def gemm_block_ops(nb):
                        """Closures emitting GEMM for ring block nb (xg cols
                        nb*512..). Returns list; caller pops a few per step."""
                        ops = []
                        ring_t = ringp.tile([128, NTIL * SBLK * BLOC], F16, tag="ring", name="ring")
                        ring_tiles[nb] = ring_t
                        xts = [None] * KC

                        def load_xt(k):
                            def _f():
                                xt = xtp.tile([128, 512], F16, tag="xt", name="xt")
                                if l == 0:
                                    pass  # resident in x0_sb; handled in mm
                                elif k < 4:
                                    nc.sync.dma_start(
                                        out=xt[:],
                                        in_=myh[k * 128 : (k + 1) * 128, nb * 512 : (nb + 1) * 512],
                                    )
                                else:
                                    j = (k - 4) * NBLK + nb
                                    nc.gpsimd.indirect_dma_start(
                                        out=xt[:],
                                        out_offset=None,
                                        in_=agout[l - 1][:],
                                        in_offset=bass.IndirectOffsetOnAxis(
                                            ap=pidx_sb[:, j : j + 1], axis=0
                                        ),
                                    )
                                xts[k] = xt
                            return _f

                        if l > 0:
                            for k in range(KC):
                                ops.append(load_xt(k))

                        psg_box = [None]

                        def mm(m, k):
                            def _f():
                                if k == 0:
                                    psg_box[0] = psgp.tile([128, 512], F32, tag="psG", name="psG")
                                if l == 0:
                                    rhs = x0_sb[:, k * TB + nb * 512 : k * TB + (nb + 1) * 512]
                                else:
                                    rhs = xts[k][:]
                                nc.tensor.matmul(
                                    psg_box[0][:],
                                    lhsT=wih_sb[:, k * G + m * 128 : k * G + (m + 1) * 128],
                                    rhs=rhs,
                                    start=(k == 0),
                                    stop=(k == KC - 1),
                                )
                            return _f

                        def evac(m, h):
                            def _f():
                                eng = evac_rr[m % 2]
                                o0, o1 = m * 512 + h * 256, m * 512 + (h + 1) * 256
                                if eng is nc.scalar:
                                    nc.scalar.activation(
                                        ring_t[:, o0:o1],
                                        psg_box[0][:, h * 256 : (h + 1) * 256],
                                        ACTF.Identity,
                                        bias=bias_sb[:, m : m + 1],
                                    )
                                else:
                                    eng.tensor_scalar_add(
                                        ring_t[:, o0:o1],
                                        psg_box[0][:, h * 256 : (h + 1) * 256],
                                        bias_sb[:, m : m + 1],
                                    )
                            return _f

                        for m in range(NTIL):
                            for k in range(KC):
                                ops.append(mm(m, k))
                            ops.append(evac(m, 0))
                            ops.append(evac(m, 1))
                        return ops

                    def chain(half, ps, c_prev, c_new, hh_t, s):
                        o = half * 32
                        tg = ewp.tile([128, 32], F32, tag=f"tg{half}")
                        nc.scalar.activation(tg[:], ps[:, 96:128], ACTF.Tanh)
                        sg = ewp.tile([128, 96], F32, tag=f"sg{half}")
                        nc.scalar.activation(sg[:], ps[:, 0:96], ACTF.Sigmoid)
                        t2 = ewp.tile([128, 32], F32, tag=f"t2{half}")
                        nc.vector.tensor_tensor(out=t2[:], in0=sg[:, 0:32], in1=tg[:], op=ALU.mult)
                        t1 = ewp.tile([128, 32], F32, tag=f"t1{half}")
                        nc.gpsimd.tensor_tensor(out=t1[:], in0=sg[:, 32:64], in1=c_prev[:, o : o + 32], op=ALU.mult)
                        nc.vector.tensor_tensor(out=c_new[:, o : o + 32], in0=t1[:], in1=t2[:], op=ALU.add)
                        tc_ = ewp.tile([128, 32], F32, tag=f"tc{half}")
                        nc.scalar.activation(tc_[:], c_new[:, o : o + 32], ACTF.Tanh)
                        for q in (0, 1):
                            nc.vector.tensor_tensor(
                                out=hh_t[:, s * 64 + o + q * 16 : s * 64 + o + q * 16 + 16],
                                in0=sg[:, 64 + q * 16 : 80 + q * 16],
                                in1=tc_[:, q * 16 : q * 16 + 16],
                                op=ALU.mult,
                            )

                    # ---- prologue: GEMM blocks 0 and 1 fully ----
                    pend = []
                    for op in gemm_block_ops(0):
                        op()
                    for op in gemm_block_ops(1):
                        op()

                    pops_per_step = max(2, (KC * NTIL + 2 * NTIL + (KC if l > 0 else 0) + SBLK - 1) // SBLK)

                    c_prev = c0
                    hh_prev = None
                    for t in range(t_len):
                        s = t % SBLK
                        nb = t // SBLK
                        if s == 0:
                            hh_tiles[nb] = hhp.tile([128, SBLK * 64], F16, tag="hh", name="hh")
                            if nb + 2 < NBLK:
                                pend = pend + gemm_block_ops(nb + 2)
                        hh_t = hh_tiles[nb]
                        ring_t = ring_tiles[nb]

                        if t == 0:
                            def hrhs(k):
                                return h0[:, k * 16 : (k + 1) * 16]
                        else:
                            hp, sp = hh_prev
                            def hrhs(k, hp=hp, sp=sp):
                                return hp[:, sp * 64 + k * 16 : sp * 64 + k * 16 + 16]

                        psA = psrp.tile([128, 128], F32, tag="psA")
                        psB = psrp.tile([128, 128], F32, tag="psB")
                        psv = {0: psA[:].rearrange("p (g m b) -> p g m b", g=4, m=2, b=16),
                               1: psB[:].rearrange("p (g m b) -> p g m b", g=4, m=2, b=16)}
                        ringv = ring_t[:].rearrange(
                            "p (g m s b) -> p g m s b", g=4, m=4, s=SBLK, b=16
                        )

                        # xg -> PSUM via identity matmul (head of each group)
                        for half in (0, 1):
                            nc.tensor.matmul(
                                psv[half][:, :, :, :],
                                lhsT=id_sb[:],
                                rhs=ringv[:, :, 2 * half : 2 * half + 2, s, :],
                                start=True,
                                stop=False,
                                skip_group_check=True,
                            )
                        # phase A: k 0,1 for all 16 tiles, k-major so each
                        # wave starts as soon as its h chunk lands
                        for k in (0, 1):
                            for mt in range(NTIL):
                                gt, j = mt // 4, mt % 4
                                dst = psv[j // 2][:, gt, j % 2, :]
                                nc.tensor.matmul(
                                    dst,
                                    lhsT=whh_sb[:, k * G + mt * 128 : k * G + (mt + 1) * 128],
                                    rhs=hrhs(k),
                                    start=False,
                                    stop=False,
                                    skip_group_check=True,
                                )
                        # phase B: k 2,3 for half-1 tiles (j in 0,1), k-major
                        for k in (2, 3):
                            for mt in range(NTIL):
                                gt, j = mt // 4, mt % 4
                                if j >= 2:
                                    continue
                                dst = psv[0][:, gt, j, :]
                                nc.tensor.matmul(
                                    dst,
                                    lhsT=whh_sb[:, k * G + mt * 128 : k * G + (mt + 1) * 128],
                                    rhs=hrhs(k),
                                    start=False,
                                    stop=(k == 3),
                                    skip_group_check=True,
                                )
                        c_new = ewp.tile([128, 64], F32, tag="c")
                        chain(0, psA, c_prev, c_new, hh_t, s)
                        # phase C: k 2,3 for half-2 tiles (j in 2,3), k-major
                        for k in (2, 3):
                            for mt in range(NTIL):
                                gt, j = mt // 4, mt % 4
                                if j < 2:
                                    continue
                                dst = psv[1][:, gt, j - 2, :]
                                nc.tensor.matmul(
                                    dst,
                                    lhsT=whh_sb[:, k * G + mt * 128 : k * G + (mt + 1) * 128],
                                    rhs=hrhs(k),
                                    start=False,
                                    stop=(k == 3),
                                    skip_group_check=True,
                                )
                        chain(1, psB, c_prev, c_new, hh_t, s)
                        for _ in range(pops_per_step):
                            if pend:
                                pend.pop(0)()
                        c_prev = c_new
                        hh_prev = (hh_t, s)

                        if l < L - 1:
                            t_rev = t_len - 1 - t
                            sdst = stage[l][:].rearrange(
                                "(nb k p) (sr b) -> p nb k sr b", nb=NBLK, k=4, sr=SBLK, b=BLOC
                            )
                            nc.sync.dma_start(
                                out=sdst[:, t_rev // SBLK, :, t_rev % SBLK, :],
                                in_=hh_t[:, s * 64 : (s + 1) * 64].rearrange("p (k b) -> p k b", k=4, b=16),
                            )
                        if s == SBLK - 1:
                            hhv = hh_t[:].rearrange("p (s k b) -> p s k b", s=SBLK, k=4, b=16)
                            for k in range(4):
                                nc.sync.dma_start(
                                    out=dst_plane[k * 128 : (k + 1) * 128,
                                                  nb * SBLK * BLOC : (nb + 1) * SBLK * BLOC],
                                    in_=hhv[:, :, k, :],
                                )
                            if l < L - 1:
                                # this step-block just completed staging block
                                # NBLK-1-nb (time-reversed); exchange it now
                                nbs = NBLK - 1 - nb
                                nc.gpsimd.collective_compute(
                                    "AllGather",
                                    ALU.bypass,
                                    replica_groups=[[0, 4], [1, 5], [2, 6], [3, 7]],
                                    ins=[stage[l][nbs * 512 : (nbs + 1) * 512, :]],
                                    outs=[agout[l][nbs * 1024 : (nbs + 1) * 1024, :]],
                                )



    _split_multi_waits(nc)
    return nc


# ----------------------------------------------------------------------------
# host side
# ----------------------------------------------------------------------------


def _prep_core_inputs(words, embed_table, params, core, t_len=T):
    d = core // 4  # 0 fwd, 1 bwd
    q = core % 4  # batch quarter
    wslice = words[q * BLOC : (q + 1) * BLOC]  # [BLOC, T]
    if d == 1:
        wslice = wslice[:, ::-1]
    x0 = embed_table[wslice]  # [BLOC, t, E]
    x0T = np.ascontiguousarray(x0.transpose(2, 1, 0)).reshape(E, t_len * BLOC)

    inp = {"x0T": x0T.astype(np.float16)}
    for l in range(L):
        w_ih, w_hh, b = params[l]
        wi = w_ih[d][_PERM].copy()  # [G, in]
        wh = w_hh[d][_PERM].copy()
        bb = b[d][_PERM].copy()
        if l > 0:
            half = np.split(wi, 2, axis=1)
            wi = np.concatenate([half[d], half[1 - d]], axis=1)  # my dir first
        inp[f"wih{l}T" if l else "wih0T"] = np.ascontiguousarray(wi.T).astype(np.float16)
        inp[f"whh{l}T"] = np.ascontiguousarray(wh.T).astype(np.float16)
        inp[f"bias{l}"] = np.ascontiguousarray(bb.reshape(NTIL, 128).T).astype(np.float32)
    nblk = t_len // SBLK
    rp = 1 - (core // 4 >= 1)  # partner slot within the 2-rank group
    pi = np.zeros((128, 4 * nblk), np.int32)
    for k in range(4):
        for nb in range(nblk):
            pi[:, k * nblk + nb] = nb * 1024 + rp * 512 + k * 128 + np.arange(128)
    inp["pidx"] = pi
    inp["ident"] = np.eye(128, dtype=np.float16)
    return inp


_NC_CACHE = {}


def _get_nc(t_len=T):
    if t_len not in _NC_CACHE:
        _NC_CACHE[t_len] = _build_nc(t_len)
    return _NC_CACHE[t_len]


def kernel(**inputs):
    words = np.asarray(inputs["words"]).astype(np.int64)
    words = np.where(words == -1, NWORDS - 1, words)
    embed_table = np.asarray(inputs["embed_table"], dtype=np.float32)
    params = []
    for l in range(L):
        params.append(
            (
                np.asarray(inputs[f"w_ih_l{l}"], dtype=np.float32),
                np.asarray(inputs[f"w_hh_l{l}"], dtype=np.float32),
                np.asarray(inputs[f"b_l{l}"], dtype=np.float32),
            )
        )

    nc = _get_nc(T)
    in_maps = [_prep_core_inputs(words, embed_table, params, c) for c in range(NCORES)]
    res = bass_utils.run_bass_kernel_spmd(nc, in_maps, core_ids=list(range(NCORES)))

    out = np.empty((B, T, 2 * H), np.float32)
    for core in range(NCORES):
        d, q = core // 4, core % 4
        ob = res.results[core]["outbuf"].astype(np.float32).reshape(4, 128, T, BLOC)  # [k, p, t, b]
        hseq = ob.transpose(3, 2, 0, 1).reshape(BLOC, T, H)  # [b, t, h]
        if d == 1:
            hseq = hseq[:, ::-1]
        out[q * BLOC : (q + 1) * BLOC, :, d * H : (d + 1) * H] = hseq
    return out


# revision 13
# speedup vs baseline: 1.1175x; 1.1175x over previous
"""Trainium2 Bass kernel for a 3-layer bidirectional LSTM encoder.

Problem: words [64,512] -> embedding [256] -> 3 x biLSTM(H=512) -> out [64,512,1024].

Sharding (8 NeuronCores): direction x batch-quarter (cores 0-3 forward, 4-7
backward on host-reversed input, 16 sequences each). v2 rewrite driven by the
baseline trace (52.4ms, PE stuck at K=4/8 half clock, fp32 2-pass matmuls at
~427ns each, 2.5us serial elementwise chain per step):

- All matmul operands fp16 (1 PE pass instead of fp32's LOW/HIGH 2-pass, and
  fast-weight-load kicks in). PSUM/c stay fp32.
- The input GEMM is interleaved between recurrence steps (a few fat N=512
  matmuls per step) so the tensor engine never idles long enough for the HAM
  clock gate to drop and the GEMM costs no extra wall time.
- xg lives in an SBUF ring (3 x 32-step blocks) - no DRAM round trip.
- xg enters PSUM through an identity matmul at the head of each accumulation
  group, so the per-step chain starts at the sigmoid (no vector add).
- Per-step chain split into two half-chains (h-chunks 0,1 | 2,3). Each half's
  sigmoid only needs its own 8 gate tiles, and next step's k01 matmuls only
  need half 1's h, so elementwise overlaps the other half's matmuls.
- tanh(g) folded into the big sigmoid via tanh(x) = 2*sigmoid(2x)-1 (g-gate
  weight rows pre-scaled 2x on host); chain ops spread over DVE/ACT/GpSimd.
- Layer exchange: pairwise AllGather (groups [q, q+4]) of fp16 staging planes;
  partner rows pulled per-GEMM-block by indirect DMA gather (host-baked row
  ids keep the SPMD program uniform).
"""

import os
import sys

import numpy as np

for _p in ("/opt/trn_rl_repo", os.path.dirname(os.path.abspath(__file__))):
    if os.path.isdir(_p) and _p not in sys.path:
        sys.path.insert(0, _p)

import bass_rust
import concourse.bass as bass
import concourse.tile as tile
from concourse import bass_utils, mybir

F32 = mybir.dt.float32
F16 = mybir.dt.float16
I32 = mybir.dt.int32
ACTF = mybir.ActivationFunctionType
ALU = mybir.AluOpType

B, T = 64, 512
NWORDS, E, H, L = 32000, 256, 512, 3
G = 4 * H  # 2048 gate rows
NCORES = 8
BLOC = B // 4  # 16 sequences per core
NTIL = G // 128  # 16 gate tiles
SBLK = 32  # recurrence steps per xg ring block (= 512 GEMM columns)

# gate-row permutation to [i f o g] tile blocks (reference order is i, f, g, o)
_PERM = np.concatenate(
    [np.arange(0, 512), np.arange(512, 1024), np.arange(1536, 2048), np.arange(1024, 1536)]
)


def _split_multi_waits(nc, max_waits=1):
    """walrus accepts at most one semaphore sync-wait per instruction; fan
    extra waits out onto same-engine NoOps just before it."""
    n = 0
    for f in nc.m.functions:
        for blk in f.blocks:
            insts = list(blk.instructions)
            out = []
            changed = False
            for inst in insts:
                si = inst.sync_info
                waits = list(si.on_wait) if si is not None else []
                if len(waits) > max_waits:
                    extra, keep = waits[:-max_waits], waits[-max_waits:]
                    for j in range(0, len(extra), max_waits):
                        nop = mybir.InstNoOp(name=f"{inst.name}-wsplit{j}", ins=[], outs=[])
                        nop.engine = inst.engine
                        nop.sync_info = bass_rust.SyncInfo(on_wait=extra[j : j + max_waits], on_update=[])
                        out.append(nop)
                        n += 1
                    inst.sync_info = bass_rust.SyncInfo(on_wait=keep, on_update=list(si.on_update))
                    changed = True
                out.append(inst)
            if changed:
                try:
                    blk.set_instructions(out)
                except Exception:
                    blk.instructions = out
    return n


def _build_nc(t_len=T):
    TB = t_len * BLOC
    NBLK = t_len // SBLK

    nc = bass.Bass(num_devices=NCORES, detect_race_conditions=False, disable_frame_to_traceback=True)

    x0T = nc.dram_tensor("x0T", [2 * 128, TB], F16, kind="ExternalInput")
    wih = [
        nc.dram_tensor("wih0T", [E, G], F16, kind="ExternalInput"),
        nc.dram_tensor("wih1T", [2 * H, G], F16, kind="ExternalInput"),
        nc.dram_tensor("wih2T", [2 * H, G], F16, kind="ExternalInput"),
    ]
    whh = [nc.dram_tensor(f"whh{l}T", [H, G], F16, kind="ExternalInput") for l in range(L)]
    bias = [nc.dram_tensor(f"bias{l}", [128, NTIL], F32, kind="ExternalInput") for l in range(L)]
    pidx = nc.dram_tensor("pidx", [128, 4 * NBLK], I32, kind="ExternalInput")
    ident = nc.dram_tensor("ident", [128, 128], F16, kind="ExternalInput")
    outbuf = nc.dram_tensor("outbuf", [4 * 128, TB], F16, kind="ExternalOutput")

    myh = nc.dram_tensor("myh", [4 * 128, TB], F16, kind="Internal")
    # stage rows = (nb, k, p): each 32-step block is a contiguous [512, 512]
    # region so its AllGather can fire as soon as the block's h is staged.
    stage = [nc.dram_tensor(f"stage{l}", [NBLK * 4 * 128, SBLK * BLOC], F16, kind="Internal") for l in range(2)]
    # agout rows = (nb, slot, k, p)
    agout = [
        nc.dram_tensor(f"agout{l}", [NBLK * 2 * 4 * 128, SBLK * BLOC], F16, kind="Internal")
        for l in range(2)
    ]

    with tile.TileContext(nc) as tc:
        with tc.tile_pool(name="const", bufs=1) as constp:
            pidx_sb = constp.tile([128, 4 * NBLK], I32, tag="pidx")
            nc.sync.dma_start(out=pidx_sb[:], in_=pidx[:])
            id_sb = constp.tile([128, 128], F16, tag="ident")
            nc.sync.dma_start(out=id_sb[:], in_=ident[:])
            x0_sb = constp.tile([128, 2 * TB], F16, tag="x0")
            for k in range(2):
                nc.sync.dma_start(out=x0_sb[:, k * TB : (k + 1) * TB], in_=x0T[k * 128 : (k + 1) * 128, :])
            h0 = constp.tile([128, 64], F16, tag="h0")
            nc.vector.memset(h0[:], 0.0)
            c0 = constp.tile([128, 64], F32, tag="c0")
            nc.vector.memset(c0[:], 0.0)

            for l in range(L):
                KC = 2 if l == 0 else 8
                dst_plane = outbuf if l == L - 1 else myh

                with tc.tile_pool(name=f"w{l}", bufs=1) as wp, \
                     tc.tile_pool(name=f"ring{l}", bufs=3) as ringp, \
                     tc.tile_pool(name=f"xt{l}", bufs=KC + 2) as xtp, \
                     tc.tile_pool(name=f"hh{l}", bufs=2) as hhp, \
                     tc.tile_pool(name=f"ew{l}", bufs=2) as ewp, \
                     tc.tile_pool(name=f"psR{l}", bufs=2, space="PSUM") as psrp, \
                     tc.tile_pool(name=f"psG{l}", bufs=2, space="PSUM") as psgp:

                    wih_sb = wp.tile([128, KC * G], F16, tag="wih")
                    for k in range(KC):
                        nc.sync.dma_start(out=wih_sb[:, k * G : (k + 1) * G], in_=wih[l][k * 128 : (k + 1) * 128, :])
                    whh_sb = wp.tile([128, 4 * G], F16, tag="whh")
                    for k in range(4):
                        nc.sync.dma_start(out=whh_sb[:, k * G : (k + 1) * G], in_=whh[l][k * 128 : (k + 1) * 128, :])
                    bias_sb = wp.tile([128, NTIL], F32, tag="bias")
                    nc.sync.dma_start(out=bias_sb[:], in_=bias[l][:])

                    ring_tiles = {}
                    hh_tiles = {}
                    evac_rr = [nc.vector, nc.vector]

                    def gemm_block_ops(nb):
                        """Closures emitting GEMM for ring block nb (xg cols
                        nb*512..). Returns list; caller pops a few per step."""
                        ops = []
                        ring_t = ringp.tile([128, NTIL * SBLK * BLOC], F16, tag="ring", name="ring")
                        ring_tiles[nb] = ring_t
                        xts = [None] * KC

                        def load_xt(k):
                            def _f():
                                xt = xtp.tile([128, 512], F16, tag="xt", name="xt")
                                if l == 0:
                                    pass  # resident in x0_sb; handled in mm
                                elif k < 4:
                                    nc.sync.dma_start(
                                        out=xt[:],
                                        in_=myh[k * 128 : (k + 1) * 128, nb * 512 : (nb + 1) * 512],
                                    )
                                else:
                                    j = (k - 4) * NBLK + nb
                                    nc.gpsimd.indirect_dma_start(
                                        out=xt[:],
                                        out_offset=None,
                                        in_=agout[l - 1][:],
                                        in_offset=bass.IndirectOffsetOnAxis(
                                            ap=pidx_sb[:, j : j + 1], axis=0
                                        ),
                                    )
                                xts[k] = xt
                            return _f

                        if l > 0:
                            for k in range(KC):
                                ops.append(load_xt(k))

                        psg_box = [None]

                        def mm(m, k):
                            def _f():
                                if k == 0:
                                    psg_box[0] = psgp.tile([128, 512], F32, tag="psG", name="psG")
                                if l == 0:
                                    rhs = x0_sb[:, k * TB + nb * 512 : k * TB + (nb + 1) * 512]
                                else:
                                    rhs = xts[k][:]
                                nc.tensor.matmul(
                                    psg_box[0][:],
                                    lhsT=wih_sb[:, k * G + m * 128 : k * G + (m + 1) * 128],
                                    rhs=rhs,
                                    start=(k == 0),
                                    stop=(k == KC - 1),
                                )
                            return _f

                        def evac(m, h):
                            def _f():
                                eng = evac_rr[m % 2]
                                o0, o1 = m * 512 + h * 256, m * 512 + (h + 1) * 256
                                if eng is nc.scalar:
                                    nc.scalar.activation(
                                        ring_t[:, o0:o1],
                                        psg_box[0][:, h * 256 : (h + 1) * 256],
                                        ACTF.Identity,
                                        bias=bias_sb[:, m : m + 1],
                                    )
                                else:
                                    eng.tensor_scalar_add(
                                        ring_t[:, o0:o1],
                                        psg_box[0][:, h * 256 : (h + 1) * 256],
                                        bias_sb[:, m : m + 1],
                                    )
                            return _f

                        for m in range(NTIL):
                            for k in range(KC):
                                ops.append(mm(m, k))
                            ops.append(evac(m, 0))
                            ops.append(evac(m, 1))
                        return ops

                    def chain(half, ps, c_prev, c_new, hh_t, s):
                        o = half * 32
                        sg = ewp.tile([128, 128], F32, tag=f"sg{half}")
                        nc.scalar.activation(sg[:], ps[:], ACTF.Sigmoid)
                        tg = ewp.tile([128, 32], F32, tag=f"tg{half}")
                        nc.vector.tensor_scalar(tg[:], sg[:, 96:128], 2.0, -1.0, ALU.mult, ALU.add)
                        t2 = ewp.tile([128, 32], F32, tag=f"t2{half}")
                        nc.vector.tensor_tensor(out=t2[:], in0=sg[:, 0:32], in1=tg[:], op=ALU.mult)
                        t1 = ewp.tile([128, 32], F32, tag=f"t1{half}")
                        nc.gpsimd.tensor_tensor(out=t1[:], in0=sg[:, 32:64], in1=c_prev[:, o : o + 32], op=ALU.mult)
                        nc.vector.tensor_tensor(out=c_new[:, o : o + 32], in0=t1[:], in1=t2[:], op=ALU.add)
                        tc_ = ewp.tile([128, 32], F32, tag=f"tc{half}")
                        nc.scalar.activation(tc_[:], c_new[:, o : o + 32], ACTF.Tanh)
                        for q in (0, 1):
                            nc.vector.tensor_tensor(
                                out=hh_t[:, s * 64 + o + q * 16 : s * 64 + o + q * 16 + 16],
                                in0=sg[:, 64 + q * 16 : 80 + q * 16],
                                in1=tc_[:, q * 16 : q * 16 + 16],
                                op=ALU.mult,
                            )

                    # ---- prologue: GEMM blocks 0 and 1 fully ----
                    pend = []
                    for op in gemm_block_ops(0):
                        op()
                    for op in gemm_block_ops(1):
                        op()

                    pops_per_step = max(2, (KC * NTIL + 2 * NTIL + (KC if l > 0 else 0) + SBLK - 1) // SBLK)

                    c_prev = c0
                    hh_prev = None
                    for t in range(t_len):
                        s = t % SBLK
                        nb = t // SBLK
                        if s == 0:
                            hh_tiles[nb] = hhp.tile([128, SBLK * 64], F16, tag="hh", name="hh")
                            if nb + 2 < NBLK:
                                pend = pend + gemm_block_ops(nb + 2)
                        hh_t = hh_tiles[nb]
                        ring_t = ring_tiles[nb]

                        if t == 0:
                            def hrhs(k):
                                return h0[:, k * 16 : (k + 1) * 16]
                        else:
                            hp, sp = hh_prev
                            def hrhs(k, hp=hp, sp=sp):
                                return hp[:, sp * 64 + k * 16 : sp * 64 + k * 16 + 16]

                        psA = psrp.tile([128, 128], F32, tag="psA")
                        psB = psrp.tile([128, 128], F32, tag="psB")
                        psv = {0: psA[:].rearrange("p (g m b) -> p g m b", g=4, m=2, b=16),
                               1: psB[:].rearrange("p (g m b) -> p g m b", g=4, m=2, b=16)}
                        ringv = ring_t[:].rearrange(
                            "p (g m s b) -> p g m s b", g=4, m=4, s=SBLK, b=16
                        )

                        # xg -> PSUM via identity matmul (head of each group)
                        for half in (0, 1):
                            nc.tensor.matmul(
                                psv[half][:, :, :, :],
                                lhsT=id_sb[:],
                                rhs=ringv[:, :, 2 * half : 2 * half + 2, s, :],
                                start=True,
                                stop=False,
                                skip_group_check=True,
                            )
                        # phase A: k 0,1 for all 16 tiles, k-major so each
                        # wave starts as soon as its h chunk lands
                        for k in (0, 1):
                            for mt in range(NTIL):
                                gt, j = mt // 4, mt % 4
                                dst = psv[j // 2][:, gt, j % 2, :]
                                nc.tensor.matmul(
                                    dst,
                                    lhsT=whh_sb[:, k * G + mt * 128 : k * G + (mt + 1) * 128],
                                    rhs=hrhs(k),
                                    start=False,
                                    stop=False,
                                    skip_group_check=True,
                                )
                        # phase B: k 2,3 for half-1 tiles (j in 0,1), k-major
                        for k in (2, 3):
                            for mt in range(NTIL):
                                gt, j = mt // 4, mt % 4
                                if j >= 2:
                                    continue
                                dst = psv[0][:, gt, j, :]
                                nc.tensor.matmul(
                                    dst,
                                    lhsT=whh_sb[:, k * G + mt * 128 : k * G + (mt + 1) * 128],
                                    rhs=hrhs(k),
                                    start=False,
                                    stop=(k == 3),
                                    skip_group_check=True,
                                )
                        c_new = ewp.tile([128, 64], F32, tag="c")
                        chain(0, psA, c_prev, c_new, hh_t, s)
                        # phase C: k 2,3 for half-2 tiles (j in 2,3), k-major
                        for k in (2, 3):
                            for mt in range(NTIL):
                                gt, j = mt // 4, mt % 4
                                if j < 2:
                                    continue
                                dst = psv[1][:, gt, j - 2, :]
                                nc.tensor.matmul(
                                    dst,
                                    lhsT=whh_sb[:, k * G + mt * 128 : k * G + (mt + 1) * 128],
                                    rhs=hrhs(k),
                                    start=False,
                                    stop=(k == 3),
                                    skip_group_check=True,
                                )
                        chain(1, psB, c_prev, c_new, hh_t, s)
                        for _ in range(pops_per_step):
                            if pend:
                                pend.pop(0)()
                        c_prev = c_new
                        hh_prev = (hh_t, s)

                        if l < L - 1:
                            t_rev = t_len - 1 - t
                            sdst = stage[l][:].rearrange(
                                "(nb k p) (sr b) -> p nb k sr b", nb=NBLK, k=4, sr=SBLK, b=BLOC
                            )
                            nc.sync.dma_start(
                                out=sdst[:, t_rev // SBLK, :, t_rev % SBLK, :],
                                in_=hh_t[:, s * 64 : (s + 1) * 64].rearrange("p (k b) -> p k b", k=4, b=16),
                            )
                        if s == SBLK - 1:
                            hhv = hh_t[:].rearrange("p (s k b) -> p s k b", s=SBLK, k=4, b=16)
                            for k in range(4):
                                nc.sync.dma_start(
                                    out=dst_plane[k * 128 : (k + 1) * 128,
                                                  nb * SBLK * BLOC : (nb + 1) * SBLK * BLOC],
                                    in_=hhv[:, :, k, :],
                                )
                            if l < L - 1:
                                # this step-block just completed staging block
                                # NBLK-1-nb (time-reversed); exchange it now
                                nbs = NBLK - 1 - nb
                                nc.gpsimd.collective_compute(
                                    "AllGather",
                                    ALU.bypass,
                                    replica_groups=[[0, 4], [1, 5], [2, 6], [3, 7]],
                                    ins=[stage[l][nbs * 512 : (nbs + 1) * 512, :]],
                                    outs=[agout[l][nbs * 1024 : (nbs + 1) * 1024, :]],
                                )



    _split_multi_waits(nc)
    return nc


# ----------------------------------------------------------------------------
# host side
# ----------------------------------------------------------------------------


def _prep_core_inputs(words, embed_table, params, core, t_len=T):
    d = core // 4  # 0 fwd, 1 bwd
    q = core % 4  # batch quarter
    wslice = words[q * BLOC : (q + 1) * BLOC]  # [BLOC, T]
    if d == 1:
        wslice = wslice[:, ::-1]
    x0 = embed_table[wslice]  # [BLOC, t, E]
    x0T = np.ascontiguousarray(x0.transpose(2, 1, 0)).reshape(E, t_len * BLOC)

    inp = {"x0T": x0T.astype(np.float16)}
    for l in range(L):
        w_ih, w_hh, b = params[l]
        wi = w_ih[d][_PERM].copy()  # [G, in]
        wh = w_hh[d][_PERM].copy()
        bb = b[d][_PERM].copy()
        # fold tanh(x) = 2*sigmoid(2x) - 1 pre-scale into the g-gate rows
        wi[1536:2048] *= 2.0
        wh[1536:2048] *= 2.0
        bb[1536:2048] *= 2.0
        if l > 0:
            half = np.split(wi, 2, axis=1)
            wi = np.concatenate([half[d], half[1 - d]], axis=1)  # my dir first
        inp[f"wih{l}T" if l else "wih0T"] = np.ascontiguousarray(wi.T).astype(np.float16)
        inp[f"whh{l}T"] = np.ascontiguousarray(wh.T).astype(np.float16)
        inp[f"bias{l}"] = np.ascontiguousarray(bb.reshape(NTIL, 128).T).astype(np.float32)
    nblk = t_len // SBLK
    rp = 1 - (core // 4 >= 1)  # partner slot within the 2-rank group
    pi = np.zeros((128, 4 * nblk), np.int32)
    for k in range(4):
        for nb in range(nblk):
            pi[:, k * nblk + nb] = nb * 1024 + rp * 512 + k * 128 + np.arange(128)
    inp["pidx"] = pi
    inp["ident"] = np.eye(128, dtype=np.float16)
    return inp


_NC_CACHE = {}


def _get_nc(t_len=T):
    if t_len not in _NC_CACHE:
        _NC_CACHE[t_len] = _build_nc(t_len)
    return _NC_CACHE[t_len]


def kernel(**inputs):
    words = np.asarray(inputs["words"]).astype(np.int64)
    words = np.where(words == -1, NWORDS - 1, words)
    embed_table = np.asarray(inputs["embed_table"], dtype=np.float32)
    params = []
    for l in range(L):
        params.append(
            (
                np.asarray(inputs[f"w_ih_l{l}"], dtype=np.float32),
                np.asarray(inputs[f"w_hh_l{l}"], dtype=np.float32),
                np.asarray(inputs[f"b_l{l}"], dtype=np.float32),
            )
        )

    nc = _get_nc(T)
    in_maps = [_prep_core_inputs(words, embed_table, params, c) for c in range(NCORES)]
    res = bass_utils.run_bass_kernel_spmd(nc, in_maps, core_ids=list(range(NCORES)))

    out = np.empty((B, T, 2 * H), np.float32)
    for core in range(NCORES):
        d, q = core // 4, core % 4
        ob = res.results[core]["outbuf"].astype(np.float32).reshape(4, 128, T, BLOC)  # [k, p, t, b]
        hseq = ob.transpose(3, 2, 0, 1).reshape(BLOC, T, H)  # [b, t, h]
        if d == 1:
            hseq = hseq[:, ::-1]
        out[q * BLOC : (q + 1) * BLOC, :, d * H : (d + 1) * H] = hseq
    return out
